# revision 11
# baseline (speedup 1.0000x reference)
# Bahdanau-style additive attention kernel for Trainium2 (8 NeuronCores).
#
#   f1 = h_j @ U_w                     [B, T, A]
#   f2 = s_t @ W_w                     [B, 1, A]
#   e  = tanh(f1 + f2) @ V_w           [B, T, 1]
#   a  = softmax(e, axis=T)            [B, T, 1]
#   c  = sum_t a * h_j                 [B, 1, ENC]
#
# Sharding: data-parallel over batch B=64 across 8 cores (8 batches/core),
# U/W/V replicated. No collectives needed.
#
# Per-core dataflow (per batch):
#   - DMA h[b] ([T, ENC]) natural layout into SBUF in 4 t-chunks.
#   - PE-transpose h -> hT chunks [ENC-part, t] (PSUM, copied to SBUF).
#   - f1^T tile [a=128, t=512] = U-chunk^T @ hT-chunk (accum over ENC/128),
#     with U as the stationary operand (float32r single-pass matmuls).
#   - ACT: g = tanh(f1^T + f2^T[a]) fused via per-partition bias.
#   - e^T chunk [1, t] = V^T @ g (accum over A/128).
#   - softmax over [1, T] (DVE max, ACT exp with fused sum, DVE recip+scale).
#   - a bounced through DRAM to get aT [t=128, 16] layout.
#   - c [1, ENC] = aT^T @ h_natural (accum over T/128).

import numpy as np

import concourse.bacc as bacc
import concourse.mybir as mybir
import concourse.tile as tile
from concourse.masks import make_identity

B, T, ENC, DEC, A = 64, 2048, 1024, 1024, 512
N_CORES = 8
BPC = B // N_CORES  # batches per core

F32 = mybir.dt.float32
F32R = mybir.dt.float32r

EO = ENC // 128  # 8  e-chunks (contraction for f1)
AO = A // 128    # 4  a-tiles
TT = T // 128    # 16 t-tiles
NTC = 4          # t-chunks per batch
TCW = T // NTC   # 512 t per chunk
TPT = TT // NTC  # 4  t-tiles per chunk
NE = ENC // 512  # 2  output free chunks for c

TANH = mybir.ActivationFunctionType.Tanh
EXP = mybir.ActivationFunctionType.Exp
AXX = mybir.AxisListType.X


def build_nc(bpc=BPC):
    nc = bacc.Bacc("TRN2", target_bir_lowering=False, debug=False)

    h = nc.dram_tensor("h", [bpc, T, ENC], F32, kind="ExternalInput")
    s = nc.dram_tensor("s", [bpc, DEC], F32, kind="ExternalInput")
    U = nc.dram_tensor("U", [ENC, A], F32, kind="ExternalInput")
    W = nc.dram_tensor("W", [DEC, A], F32, kind="ExternalInput")
    Vw = nc.dram_tensor("Vw", [A, 1], F32, kind="ExternalInput")
    c_out = nc.dram_tensor("c_out", [bpc, ENC], F32, kind="ExternalOutput")
    a_out = nc.dram_tensor("a_out", [bpc, T], F32, kind="ExternalOutput")

    with tile.TileContext(nc) as tc:
        with (
            tc.tile_pool(name="consts", bufs=1) as consts,
            tc.tile_pool(name="hpool", bufs=6) as hpool,
            tc.tile_pool(name="htp", bufs=2) as htp,
            tc.tile_pool(name="gpool", bufs=2) as gpool,
            tc.tile_pool(name="small", bufs=2) as small,
            tc.tile_pool(name="ptr", bufs=2, space="PSUM") as ptr_pool,
            tc.tile_pool(name="pf1", bufs=2, space="PSUM") as pf1_pool,
            tc.tile_pool(name="pe", bufs=2, space="PSUM") as pe_pool,
            tc.tile_pool(name="pc", bufs=1, space="PSUM") as pc_pool,
        ):
            ident32 = consts.tile([128, 128], F32, name="ident32")
            make_identity(nc, ident32)
            ident = consts.tile([128, 128], F32R, name="ident")
            nc.vector.tensor_copy(ident[:], ident32[:])

            U_sb = consts.tile([128, EO, A], F32R, name="U_sb")
            nc.gpsimd.dma_start(U_sb[:], U.ap().rearrange("(eo p) a -> p eo a", p=128))
            V_sb = consts.tile([128, AO], F32R, name="V_sb")
            with nc.allow_non_contiguous_dma(reason="tiny V load"):
                nc.gpsimd.dma_start(
                    V_sb[:], Vw.ap().rearrange("(ao p) one -> p (ao one)", p=128)
                )
            f2T = consts.tile([128, AO, bpc], F32, name="f2T")

            # --- setup: f2^T[a, b] = (s @ W)^T, computed once ---
            with tc.tile_pool(name="setup", bufs=1) as setup:
                W_sb = setup.tile([128, EO, A], F32R, name="W_sb")
                nc.gpsimd.dma_start(
                    W_sb[:], W.ap().rearrange("(eo p) a -> p eo a", p=128)
                )
                s_sb = setup.tile([128, DEC], F32R, name="s_sb")
                nc.vector.memset(s_sb[:].bitcast(F32), 0.0)
                nc.gpsimd.dma_start(s_sb[:bpc, :], s.ap())
                sT_sb = setup.tile([128, EO, bpc], F32R, name="sT_sb")
                for ec in range(EO):
                    pst = ptr_pool.tile([128, TCW], F32R, tag="ptr", name="pst")[
                        :, :128
                    ]
                    nc.tensor.transpose(
                        pst[:], s_sb[:, ec * 128 : (ec + 1) * 128], ident[:]
                    )
                    nc.vector.tensor_copy(sT_sb[:, ec, :], pst[:, :bpc])
                for at in range(AO):
                    pf2 = pf1_pool.tile([128, TCW], F32, tag="pf1", name="pf2")[
                        :, :bpc
                    ]
                    for ec in range(EO):
                        nc.tensor.matmul(
                            pf2[:],
                            W_sb[:, ec, at * 128 : (at + 1) * 128],
                            sT_sb[:, ec, :],
                            start=(ec == 0),
                            stop=(ec == EO - 1),
                        )
                    nc.vector.tensor_copy(f2T[:, at, :], pf2[:])

            # --- main per-batch pipeline ---
            pend_e = None     # deferred e-matvec emission (one t-chunk back)
            pend_tail = None  # deferred c-matmul emission (one batch back)

            for b in range(bpc):
                h_tiles = []
                for tci in range(NTC):
                    ht = hpool.tile([128, TPT, ENC], F32R, tag="h", name="h_sb")
                    nc.gpsimd.dma_start(
                        ht[:],
                        h.ap()[b, tci * TCW : (tci + 1) * TCW, :].rearrange(
                            "(tt p) e -> p tt e", p=128
                        ),
                    )
                    h_tiles.append(ht)

                e_sb = small.tile([1, T], F32, tag="e", name="e_sb")

                for tci in range(NTC):
                    # transpose h chunk -> hT [e-part, EO, t]
                    hT = htp.tile([128, EO, TCW], F32R, tag="hT", name="hT")
                    for ec in range(EO):
                        ptr = ptr_pool.tile([128, TCW], F32R, tag="ptr", name="ptr")
                        for tt in range(TPT):
                            nc.tensor.transpose(
                                ptr[:, tt * 128 : (tt + 1) * 128],
                                h_tiles[tci][:, tt, ec * 128 : (ec + 1) * 128],
                                ident[:],
                            )
                        if ec % 2 == 0:
                            nc.scalar.copy(hT[:, ec, :], ptr[:])
                        else:
                            nc.vector.tensor_copy(hT[:, ec, :], ptr[:])

                    # f1^T tiles + fused tanh(+f2) -> g
                    g = gpool.tile([128, AO, TCW], F32R, tag="g", name="g_sb")
                    for at in range(AO):
                        pf1 = pf1_pool.tile([128, TCW], F32, tag="pf1", name="pf1")
                        for ec in range(EO):
                            nc.tensor.matmul(
                                pf1[:],
                                U_sb[:, ec, at * 128 : (at + 1) * 128],
                                hT[:, ec, :],
                                start=(ec == 0),
                                stop=(ec == EO - 1),
                            )
                        nc.scalar.activation(
                            g[:, at, :], pf1[:], TANH, bias=f2T[:, at, b : b + 1]
                        )

                    # deferred emissions to keep the PE stream dense
                    if pend_tail is not None and tci == 1:
                        pend_tail()
                        pend_tail = None
                    if pend_e is not None:
                        pend_e()

                    def mk_e(g=g, tci=tci, e_sb=e_sb):
                        def go():
                            pe = pe_pool.tile([128, TCW], F32, tag="pe", name="pe")
                            for at in range(AO):
                                nc.tensor.matmul(
                                    pe[:1, :],
                                    V_sb[:, at : at + 1],
                                    g[:, at, :],
                                    start=(at == 0),
                                    stop=(at == AO - 1),
                                )
                            nc.vector.tensor_copy(
                                e_sb[:, tci * TCW : (tci + 1) * TCW], pe[:1, :]
                            )

                        return go

                    pend_e = mk_e()

                pend_e()
                pend_e = None

                # softmax over [1, T] (in place: e -> exp(e - max) -> a)
                nmax = small.tile([1, 1], F32, tag="nmax", name="nmax")
                nc.vector.reduce_max(nmax[:], e_sb[:], axis=AXX, negate=True)
                ssum = small.tile([1, 1], F32, tag="ssum", name="ssum")
                nc.scalar.activation(
                    e_sb[:], e_sb[:], EXP, bias=nmax[:], accum_out=ssum[:]
                )
                rs = small.tile([1, 1], F32, tag="rs", name="rs")
                nc.vector.reciprocal(rs[:], ssum[:])
                nc.vector.tensor_scalar_mul(e_sb[:], e_sb[:], rs[:])
                nc.sync.dma_start(a_out.ap()[b : b + 1, :], e_sb[:])

                # bounce a through a_out (DRAM) to get the [t=128, TT] layout
                aT_sb = small.tile([128, TT], F32R, tag="aT", name="aT_sb")
                with nc.allow_non_contiguous_dma(reason="tiny aT gather"):
                    nc.gpsimd.dma_start(
                        aT_sb[:],
                        a_out.ap()[b, :].rearrange("(tt p) -> p tt", p=128),
                    )

                def mk_tail(b=b, aT_sb=aT_sb, h_tiles=h_tiles):
                    def go():
                        pc = pc_pool.tile([128, ENC], F32, tag="pc", name="pc")
                        for n in range(NE):
                            for tt in range(TT):
                                tci, tloc = divmod(tt, TPT)
                                nc.tensor.matmul(
                                    pc[:1, n * 512 : (n + 1) * 512],
                                    aT_sb[:, tt : tt + 1],
                                    h_tiles[tci][:, tloc, n * 512 : (n + 1) * 512],
                                    start=(tt == 0),
                                    stop=(tt == TT - 1),
                                )
                        c_sb = small.tile([1, ENC], F32, tag="c", name="c_sb")
                        nc.scalar.copy(c_sb[:], pc[:1, :])
                        nc.sync.dma_start(c_out.ap()[b : b + 1, :], c_sb[:])

                    return go

                pend_tail = mk_tail()

            pend_tail()

    nc.compile()
    return nc


_NC_CACHE = {}


def _get_nc(bpc=BPC):
    if bpc not in _NC_CACHE:
        _NC_CACHE[bpc] = build_nc(bpc)
    return _NC_CACHE[bpc]


def kernel(h_j, s_t, U_w, W_w, V_w):
    from concourse.bass_utils import run_bass_kernel_spmd

    h_j = np.asarray(h_j, dtype=np.float32)
    s_t = np.asarray(s_t, dtype=np.float32)
    U_w = np.asarray(U_w, dtype=np.float32)
    W_w = np.asarray(W_w, dtype=np.float32)
    V_w = np.asarray(V_w, dtype=np.float32)

    nc = _get_nc()
    in_maps = [
        {
            "h": h_j[i * BPC : (i + 1) * BPC],
            "s": s_t[i * BPC : (i + 1) * BPC],
            "U": U_w,
            "W": W_w,
            "Vw": V_w,
        }
        for i in range(N_CORES)
    ]
    res = run_bass_kernel_spmd(nc, in_maps, list(range(N_CORES))).results
    c = np.concatenate([res[i]["c_out"] for i in range(N_CORES)], axis=0)
    a = np.concatenate([res[i]["a_out"] for i in range(N_CORES)], axis=0)
    return (c[:, None, :].astype(np.float32), a[:, :, None].astype(np.float32))
